# revision 29
# baseline (speedup 1.0000x reference)
"""Trainium2 Bass kernel for ManualCausalSelfAttention.

Full (unsharded) inputs -> full output. Internally shards across 8
NeuronCores: data-parallel over batch (4) x tensor-parallel over head
groups of 8 heads (2). Each core computes a partial output projection
(its 512 rows of W_proj); the host sums the two partials per batch.

v2 design (vs baseline):
  - all matmul operands bf16 (psum stays f32): halves DMA/SBUF/LDW and
    unlocks DVE 2x modes; fp32r was already 1 cyc/row at N>=256 so PE
    cycles are unchanged.
  - single x streaming pass: q/k projection + RoPE and v projection
    share the same xt tile loop.
  - psum->sbuf bias-add + bf16 cast on the (otherwise idle) Scalar
    engine via activation(Identity, bias=per-partition).
  - causal mask without gpsimd affine_select: sub-diagonal key tiles are
    skipped outright (valid-region ST/exp/PV), and the 128-wide diagonal
    blocks get a -480 bias added in PSUM via a tiny identity x maskbias
    matmul before the exp (exp scale 1/8 turns it into -60).
  - out-proj psum tiles share the ST pool so PSUM fits in 8 banks with
    double-buffered attention pipelines on both the ST and PV side.
"""

import functools
import os
import sys

import numpy as np

for _p in (
    "/root/.axon_site",
    "/root/.axon_site/_ro/trn_rl_repo",
    "/root/.axon_site/_ro/pypackages",
    "/opt/trn_rl_repo",
):
    if os.path.isdir(_p) and _p not in sys.path:
        sys.path.append(_p)

import concourse.bass as bass
import concourse.mybir as mybir
import concourse.tile as tile
from concourse.bass_utils import run_bass_kernel_spmd

# The PJRT boundary swallows python exceptions raised by the bass compile
# hook ("CallFunctionObjArgs: error condition !(py_result)"); wrap the hook
# so the real compile error is printed before being re-raised.
from concourse import bass2jax as _b2j

if not getattr(_b2j, "_hook_wrapped", False):
    _orig_hook = _b2j.neuronx_cc_hook

    def _loud_hook(*a, **k):
        try:
            return _orig_hook(*a, **k)
        except BaseException:
            import traceback

            traceback.print_exc()
            raise

    _b2j.neuronx_cc_hook = _loud_hook
    _b2j._hook_wrapped = True

HID = 1024
NH_GLOBAL = 16
NHL = 8  # heads per core
D = 64
THETA = 10000.0
PAD_NEG = -60.0  # exp(scale*logit + PAD_NEG) == 0 relative to real keys
MASK_NEG = -480.0  # pre-scale causal bias: -480/8 == -60
F32 = mybir.dt.float32
BF16 = mybir.dt.bfloat16
ALU = mybir.AluOpType
AFT = mybir.ActivationFunctionType

TB = 512  # token block (phase 1, matmul N)
QB = 512  # query block (phase 2)

# stream_shuffle mask swapping adjacent partition pairs (per 32-group)
PAIR_SWAP_MASK = [i + 1 - 2 * (i % 2) for i in range(32)]


def _split_sync_waits(nc, max_waits=1):
    """walrus in this environment rejects instructions carrying more than
    `max_waits` sem waits ("Too many sync wait commands"); Tile's kernel-tail
    drain carries one wait per logical proc. Split the excess waits onto
    preceding same-engine NOPs."""
    for fn in nc.m.functions:
        for blk in fn.blocks:
            if not any(
                ins.sync_info is not None
                and ins.sync_info.on_wait
                and len(ins.sync_info.on_wait)
                > (0 if isinstance(ins, mybir.InstDrain) else max_waits)
                for ins in blk.instructions
            ):
                continue
            new_insts = []
            for ins in blk.instructions:
                si = ins.sync_info
                limit = 0 if isinstance(ins, mybir.InstDrain) else max_waits
                if si is not None and si.on_wait and len(si.on_wait) > limit:
                    waits = list(si.on_wait)
                    if limit:
                        extra, keep = waits[:-limit], waits[-limit:]
                    else:
                        extra, keep = waits, []
                    for i in range(0, len(extra), max_waits):
                        new_insts.append(
                            mybir.InstNoOp(
                                name=f"{ins.name}-sw{i}",
                                sync_info=mybir.SyncInfo(
                                    on_wait=extra[i : i + max_waits], on_update=[]
                                ),
                                bass_nofuse=True,
                                engine=ins.engine,
                            )
                        )
                    ins.sync_info = mybir.SyncInfo(
                        on_wait=keep, on_update=list(si.on_update)
                    )
                new_insts.append(ins)
            blk.instructions[:] = new_insts


def build_kernel(s=2048, split_waits=True, zero_bias=False):
    """Build the per-core Bass module (same program on all 8 cores).

    split_waits applies the walrus wait-limit workaround; disable it when
    running under CoreSim (the sim rejects the unregistered NOPs).
    zero_bias skips the bias matmuls (selected host-side when the actual
    b_qkv/b_proj inputs are all zero, as in this problem's setup).
    """
    nc = bass.Bass()
    nb = s // TB      # token blocks
    nqb = s // QB     # query blocks
    nkt = s // 128    # 128-wide key tiles / token tiles
    hkt = HID // 128  # hidden contraction tiles

    xT = nc.dram_tensor("xt", [HID, s], BF16, kind="ExternalInput")
    w_qk = nc.dram_tensor("w_qk", [HID, 2 * NHL * D], BF16, kind="ExternalInput")
    w_v = nc.dram_tensor("w_v", [HID, NHL * D], BF16, kind="ExternalInput")
    w_pr = nc.dram_tensor("w_proj", [NHL * D, HID], BF16, kind="ExternalInput")
    cos_d = nc.dram_tensor("cos_t", [128, s], BF16, kind="ExternalInput")
    sin_d = nc.dram_tensor("sin_t", [128, s], BF16, kind="ExternalInput")
    bqk_d = nc.dram_tensor("b_qk", [128, 8], F32, kind="ExternalInput")
    bv_d = nc.dram_tensor("b_v", [1, NHL * D], BF16, kind="ExternalInput")
    bpr_d = nc.dram_tensor("b_proj", [1, HID], BF16, kind="ExternalInput")
    padm_d = nc.dram_tensor("pad_mul", [128, nkt], F32, kind="ExternalInput")
    padb_d = nc.dram_tensor("pad_bias", [128, nkt], F32, kind="ExternalInput")
    ones_d = nc.dram_tensor("ones_in", [128, 128], BF16, kind="ExternalInput")
    ident_d = nc.dram_tensor("ident", [128, 128], BF16, kind="ExternalInput")
    maskb_d = nc.dram_tensor("maskb", [128, 128], BF16, kind="ExternalInput")
    z_d = nc.dram_tensor("z", [s, HID], F32, kind="ExternalOutput")
    # DRAM bounce buffers: transpose denominators onto 128 partitions for an
    # efficient exact reciprocal, then broadcast the result across partitions.
    den_scr = nc.dram_tensor("den_scr", [nqb * 4, 2 * QB], F32)
    rcp_scr = nc.dram_tensor("rcp_scr", [nqb * 4, 2 * QB], F32)

    with tile.TileContext(nc) as tc:
        with (
            tc.tile_pool(name="persist", bufs=1) as ppool,
            tc.tile_pool(name="qkstore", bufs=1) as qkpool,
            tc.tile_pool(name="vstore", bufs=1) as vpool,
        ):
            padm_sb = ppool.tile([128, nkt], F32, tag="padm")
            padb_sb = ppool.tile([128, nkt], F32, tag="padb")
            ones_sb = ppool.tile([1, 128], BF16, tag="ones")
            ident_sb = ppool.tile([128, 128], BF16, tag="ident")
            maskb_sb = ppool.tile([128, 128], BF16, tag="maskb")
            nc.sync.dma_start(padm_sb[:], padm_d[:])
            nc.sync.dma_start(padb_sb[:], padb_d[:])
            nc.sync.dma_start(ones_sb[:], ones_d[0:1, 0:128])
            nc.sync.dma_start(ident_sb[:], ident_d[:])
            nc.sync.dma_start(maskb_sb[:], maskb_d[:])

            # q'T / k'T (RoPE'd, feature-major): 8 tiles of [128, s];
            # tiles 0..3 = Q head-pairs, 4..7 = K head-pairs.
            qk_store = [
                qkpool.tile([128, s], BF16, name=f"qk{mt}", tag=f"qk{mt}")
                for mt in range(8)
            ]
            # V with the denominator-ones column appended per head. A DMA
            # here would need 16k 2-byte descriptors; memset is ~free.
            v_sb = vpool.tile([128, nkt, NHL, D + 1], BF16, tag="v")
            nc.vector.memset(v_sb[:, :, :, D : D + 1], 1.0)

            # ------- Single software pipeline: per token block nbi, do the
            # q/k/v projection + RoPE for that block, then the attention for
            # query block qb==nbi (causality: it only needs k/v blocks <=
            # nbi), then the out-projection for qb-1 (one block behind so
            # the softmax-denominator DRAM bounce never stalls the PE).
            with (
                tc.tile_pool(name="xt", bufs=2) as xpool,
                tc.tile_pool(name="c1", bufs=1) as c1pool,
                tc.tile_pool(name="ps_big", bufs=2, space="PSUM") as psBig,
                tc.tile_pool(name="ps_pv", bufs=2, space="PSUM") as psPV,
                tc.tile_pool(name="rope", bufs=4) as rpool,
                tc.tile_pool(name="pt", bufs=4) as ptpool,
                tc.tile_pool(name="ytn", bufs=8) as ytnpool,
                tc.tile_pool(name="small", bufs=3) as smpool,
                tc.tile_pool(name="zs", bufs=3) as zspool,
            ):
                w_qk_sb = c1pool.tile([128, hkt, 2 * NHL * D], BF16, tag="wqk")
                w_v_sb = c1pool.tile([128, hkt, NHL * D], BF16, tag="wv")
                cos_sb = c1pool.tile([128, s], BF16, tag="cos")
                sin_sb = c1pool.tile([128, s], BF16, tag="sin")
                bqk_sb = c1pool.tile([128, 8], F32, tag="bqk")
                bv_sb = c1pool.tile([1, NHL * D], BF16, tag="bv")
                w_pr_sb = c1pool.tile([128, NHL * D // 128, HID], BF16, tag="wpr")
                bpr_sb = c1pool.tile([1, HID], BF16, tag="bpr")
                xT_r = xT.rearrange("(kt p) t -> p kt t", p=128)
                xt0 = xpool.tile([128, hkt, TB], BF16, tag="xt")
                for i in range(4):
                    nc.sync.dma_start(
                        xt0[:, 2 * i : 2 * i + 2, :], xT_r[:, 2 * i : 2 * i + 2, 0:TB]
                    )
                # split the big weight DMA so several queues run in parallel;
                # column-group order matches the mt emission order (k first)
                w_qk_r = w_qk.rearrange("(kt p) f -> p kt f", p=128)
                for c in (2, 3, 0, 1):
                    nc.sync.dma_start(
                        w_qk_sb[:, :, c * 256 : (c + 1) * 256],
                        w_qk_r[:, :, c * 256 : (c + 1) * 256],
                    )
                nc.sync.dma_start(
                    w_v_sb[:], w_v.rearrange("(kt p) f -> p kt f", p=128)
                )
                nc.sync.dma_start(cos_sb[:], cos_d[:])
                nc.sync.dma_start(sin_sb[:], sin_d[:])
                nc.sync.dma_start(bqk_sb[:], bqk_d[:])
                nc.sync.dma_start(bv_sb[:], bv_d[:])
                nc.sync.dma_start(
                    w_pr_sb[:], w_pr.rearrange("(sl p) f -> p sl f", p=128)
                )
                nc.sync.dma_start(bpr_sb[:], bpr_d[:])

                def emit_proj(nbi, xt):
                    tsl = slice(nbi * TB, (nbi + 1) * TB)
                    # k head-pairs first (mt 4..7): the first ST matmuls of
                    # this query block depend on k of every block plus q of
                    # pair 0, so this order lets attention start while the
                    # DVE still applies RoPE to q pairs 1-3.
                    for mt in (4, 5, 6, 7, 0, 1, 2, 3):
                        ps = psBig.tile([128, 2 * QB], F32, tag="big")
                        for kt in range(hkt):
                            nc.tensor.matmul(
                                ps[:, 0:TB],
                                w_qk_sb[:, kt, mt * 128 : (mt + 1) * 128],
                                xt[:, kt, :],
                                start=(kt == 0),
                                stop=(kt == hkt - 1),
                            )
                        # psum -> sbuf bf16 with per-partition bias on ScalarE
                        qraw = rpool.tile([128, TB], BF16, tag="qraw")
                        nc.scalar.activation(
                            qraw[:], ps[:, 0:TB], AFT.Identity,
                            bias=0.0 if zero_bias else bqk_sb[:, mt : mt + 1],
                        )
                        t1 = rpool.tile([128, TB], BF16, tag="t1")
                        t2p = rpool.tile([128, TB], BF16, tag="t2p")
                        t2 = rpool.tile([128, TB], BF16, tag="t2")
                        nc.vector.tensor_mul(t1[:], qraw[:], cos_sb[:, tsl])
                        # pair-swapping (qraw*sin_pre) gives rotate_half(qraw)
                        # * sin (sign is in the table: +sin even rows, -sin
                        # odd rows).
                        nc.vector.tensor_mul(t2p[:], qraw[:], sin_sb[:, tsl])
                        nc.vector.stream_shuffle(t2[:], t2p[:], PAIR_SWAP_MASK)
                        nc.vector.tensor_add(qk_store[mt][:, tsl], t1[:], t2[:])
                    for vt in range(TB // 128):
                        psv = psBig.tile([128, 2 * QB], F32, tag="big")
                        for kt in range(hkt):
                            nc.tensor.matmul(
                                psv[:, 0 : NHL * D],
                                xt[:, kt, vt * 128 : (vt + 1) * 128],
                                w_v_sb[:, kt, :],
                                start=(kt == 0),
                                stop=(zero_bias and kt == hkt - 1),
                            )
                        if not zero_bias:
                            nc.tensor.matmul(
                                psv[:, 0 : NHL * D],
                                ones_sb[:],
                                bv_sb[:],
                                start=False,
                                stop=True,
                            )
                        ktix = nbi * (TB // 128) + vt
                        # psum -> sbuf bf16 cast on ScalarE
                        nc.scalar.activation(
                            v_sb[:, ktix, :, 0:D],
                            psv[:, 0 : NHL * D].rearrange("p (h d) -> p h d", d=D),
                            AFT.Identity,
                        )

                def emit_tt_head(qb, ytns, tt):
                    zp = psBig.tile([128, 2 * QB], F32, tag="big")
                    for ob in range(HID // 512):
                        for pair in range(3):
                            nc.tensor.matmul(
                                zp[:, ob * 512 : (ob + 1) * 512],
                                ytns[pair][:, tt * 128 : (tt + 1) * 128],
                                w_pr_sb[:, pair, ob * 512 : (ob + 1) * 512],
                                start=(pair == 0),
                                stop=False,
                            )
                    return zp

                def emit_tt_tail(qb, ytns, tt, zp):
                    gt = qb * (QB // 128) + tt
                    for ob in range(HID // 512):
                        nc.tensor.matmul(
                            zp[:, ob * 512 : (ob + 1) * 512],
                            ytns[3][:, tt * 128 : (tt + 1) * 128],
                            w_pr_sb[:, 3, ob * 512 : (ob + 1) * 512],
                            start=False,
                            stop=zero_bias,
                        )
                        if not zero_bias:
                            nc.tensor.matmul(
                                zp[:, ob * 512 : (ob + 1) * 512],
                                ones_sb[:],
                                bpr_sb[:, ob * 512 : (ob + 1) * 512],
                                start=False,
                                stop=True,
                            )
                    zs = zspool.tile([128, HID], F32, tag="zs")
                    nc.vector.tensor_scalar(
                        out=zs[:],
                        in0=zp[:],
                        scalar1=padm_sb[:, gt : gt + 1],
                        scalar2=None,
                        op0=ALU.mult,
                    )
                    nc.sync.dma_start(z_d[gt * 128 : (gt + 1) * 128, :], zs[:])

                def emit_outproj(qb, ytns, batch_tail=False):
                    # out-projection for query block qb (runs one qb behind
                    # attention so the den-reciprocal DRAM bounce never
                    # stalls the in-order PE). batch_tail (final block only,
                    # where this IS the critical path): emit the pair-0..2
                    # matmuls of two token tiles before any pair-3 matmul,
                    # so most work issues while pair 3's normalization is
                    # still in flight.
                    if not batch_tail:
                        for tt in range(QB // 128):
                            zp = emit_tt_head(qb, ytns, tt)
                            emit_tt_tail(qb, ytns, tt, zp)
                    else:
                        for tg in range(0, QB // 128, 2):
                            zp0 = emit_tt_head(qb, ytns, tg)
                            zp1 = emit_tt_head(qb, ytns, tg + 1)
                            emit_tt_tail(qb, ytns, tg, zp0)
                            emit_tt_tail(qb, ytns, tg + 1, zp1)

                prev = None
                xt_next = xt0
                for nbi in range(nb):
                    qb = nbi
                    xt = xt_next
                    emit_proj(nbi, xt)
                    if nbi + 1 < nb:
                        # prefetch next token block while attention runs
                        xt_next = xpool.tile([128, hkt, TB], BF16, tag="xt")
                        nc.sync.dma_start(
                            xt_next[:],
                            xT_r[:, :, (nbi + 1) * TB : (nbi + 2) * TB],
                        )
                    nkts = (qb + 1) * (QB // 128)
                    ytns = []
                    for pair in range(4):
                        qst = qk_store[pair]
                        kst = qk_store[4 + pair]
                        yp = psPV.tile([D + 1, 2 * QB], F32, name="pv", tag="pv")
                        dstart = qb * (QB // 128)
                        for ki, kt in enumerate(range(nkts)):
                            r = kt - dstart  # >=0 on diagonal group
                            off = max(r, 0) * 128
                            stp = psBig.tile([128, 2 * QB], F32, tag="big")
                            for h2 in (0, 1):
                                lo = h2 * 64
                                nc.tensor.matmul(
                                    stp[:, h2 * QB + off : (h2 + 1) * QB],
                                    kst[lo : lo + 64, kt * 128 : (kt + 1) * 128],
                                    qst[lo : lo + 64, qb * QB + off : (qb + 1) * QB],
                                    start=True,
                                    stop=(r < 0),
                                    tile_position=(lo, 0),
                                )
                                if r >= 0:
                                    # diagonal 128-block: add -480 causal bias
                                    nc.tensor.matmul(
                                        stp[:, h2 * QB + off : h2 * QB + off + 128],
                                        ident_sb[:],
                                        maskb_sb[:],
                                        start=False,
                                        stop=True,
                                    )
                            # P = exp(logit/8 + pad_bias_k) on the valid cols
                            pt = ptpool.tile([128, 2, QB], BF16, tag="pt")
                            nc.scalar.activation(
                                pt[:, :, off:],
                                stp.rearrange("p (h q) -> p h q", h=2)[:, :, off:],
                                AFT.Exp,
                                bias=padb_sb[:, kt : kt + 1],
                                scale=float(D) ** -0.5,
                            )
                            for h2 in (0, 1):
                                head = pair * 2 + h2
                                nc.tensor.matmul(
                                    yp[:, h2 * QB + off : (h2 + 1) * QB],
                                    v_sb[:, kt, head, :],
                                    pt[:, h2, off:],
                                    start=(ki == 0),
                                    stop=(ki == nkts - 1),
                                )
                        # Softmax denominators: bounce the accumulator row
                        # through DRAM to spread it over 128 partitions, take
                        # an exact DVE reciprocal, bounce back, and broadcast
                        # per head.
                        row = qb * 4 + pair
                        den = smpool.tile([1, 2 * QB], F32, name="den", tag="den")
                        nc.vector.tensor_copy(den[:], yp[D : D + 1, :])
                        nc.gpsimd.dma_start(den_scr[row : row + 1, :], den[:])
                        dent = smpool.tile(
                            [128, 2 * QB // 128], F32, name="dent", tag="dent"
                        )
                        nc.gpsimd.dma_start(
                            dent[:],
                            den_scr[row : row + 1, :].rearrange(
                                "o (p f) -> (o p) f", p=128
                            ),
                        )
                        rcpt = smpool.tile(
                            [128, 2 * QB // 128], F32, name="rcpt", tag="rcpt"
                        )
                        nc.vector.reciprocal(rcpt[:], dent[:])
                        nc.gpsimd.dma_start(
                            rcp_scr[row : row + 1, :].rearrange(
                                "o (p f) -> (o p) f", p=128
                            ),
                            rcpt[:],
                        )
                        ytn = ytnpool.tile([128, QB], BF16, tag="ytn")
                        for h2 in (0, 1):
                            rb = smpool.tile([64, QB], F32, tag="rb")
                            nc.gpsimd.dma_start(
                                rb[:],
                                rcp_scr[
                                    row : row + 1, h2 * QB : (h2 + 1) * QB
                                ].broadcast_to([64, QB]),
                            )
                            nc.vector.tensor_mul(
                                ytn[h2 * 64 : (h2 + 1) * 64, :],
                                yp[0:D, h2 * QB : (h2 + 1) * QB],
                                rb[:],
                            )
                        ytns.append(ytn)
                        if pair == 1 and prev is not None:
                            # out-proj of the previous query block, emitted
                            # mid-attention: its operands are long ready, so
                            # it fills PE while the exp pipeline works.
                            emit_outproj(qb - 1, prev)
                    prev = ytns
                emit_outproj(nqb - 1, prev, batch_tail=True)

                # (loop above assumes one query block per token block)
                assert nb == nqb
    if split_waits:
        _split_sync_waits(nc)
    return nc


@functools.lru_cache(maxsize=2)
def _built(s, zero_bias=False):
    return build_kernel(s, zero_bias=zero_bias)


def _rope_tables(s):
    j = np.arange(D // 2, dtype=np.float64)
    inv = THETA ** (-2.0 * j / D)
    ang = np.arange(s, dtype=np.float64)[:, None] * inv[None, :]  # [s, 32]
    cos = np.cos(ang).T  # [32, s]
    sin = np.sin(ang).T
    cos64 = np.repeat(cos, 2, axis=0)  # rows 2j, 2j+1 identical
    sin64 = np.repeat(sin, 2, axis=0)
    # "pre-swap" sign convention: the kernel multiplies by this table BEFORE
    # pair-swapping partitions, so odd rows carry the minus sign.
    sin64[1::2, :] *= -1.0
    cos128 = np.concatenate([cos64, cos64], axis=0)
    sin128 = np.concatenate([sin64, sin64], axis=0)
    return np.ascontiguousarray(cos128), np.ascontiguousarray(sin128)


def _col_tiled(vec, tile_rows=128):
    """[n] -> [tile_rows, n//tile_rows], column t = vec[t*128:(t+1)*128]."""
    n = vec.shape[0]
    return np.ascontiguousarray(vec.reshape(n // tile_rows, tile_rows).T)


def _bf16(a):
    import ml_dtypes

    return np.ascontiguousarray(np.asarray(a).astype(ml_dtypes.bfloat16))


def make_in_maps(x, attention_padding, W_qkv, b_qkv, W_proj, b_proj):
    x = np.asarray(x, dtype=np.float32)
    pad = np.asarray(attention_padding).astype(bool)
    W_qkv = np.asarray(W_qkv, dtype=np.float32)
    b_qkv = np.asarray(b_qkv, dtype=np.float32)
    W_proj = np.asarray(W_proj, dtype=np.float32)
    b_proj = np.asarray(b_proj, dtype=np.float32)
    B, s, _ = x.shape
    cos128, sin128 = _rope_tables(s)
    cos128_bf = _bf16(cos128)
    sin128_bf = _bf16(sin128)
    ident = np.eye(128, dtype=np.float32)
    ones = np.ones((128, 128), dtype=np.float32)
    maskb = np.where(
        np.arange(128)[None, :] < np.arange(128)[:, None], MASK_NEG, 0.0
    ).astype(np.float32)

    per_hp = {}
    for hp in range(2):
        hs = slice(hp * NHL * D, (hp + 1) * NHL * D)
        Wq = W_qkv[:, 0:HID][:, hs]
        Wk = W_qkv[:, HID : 2 * HID][:, hs]
        Wv = W_qkv[:, 2 * HID : 3 * HID][:, hs]
        bq = b_qkv[0:HID][hs]
        bk = b_qkv[HID : 2 * HID][hs]
        bv = b_qkv[2 * HID : 3 * HID][hs]
        bqk = np.concatenate([bq, bk])
        per_hp[hp] = dict(
            w_qk=_bf16(np.concatenate([Wq, Wk], axis=1)),
            w_v=_bf16(Wv),
            w_proj=_bf16(W_proj[hs, :]),
            b_qk=_col_tiled(bqk),
            b_v=_bf16(bv[None, :]),
            b_proj=_bf16(
                (b_proj if hp == 0 else np.zeros_like(b_proj))[None, :]
            ),
        )

    per_b = {}
    for b in range(B):
        p = pad[b].astype(np.float32)
        per_b[b] = dict(
            xt=_bf16(x[b].T),
            pad_mul=_col_tiled(p),
            pad_bias=_col_tiled(np.where(pad[b], 0.0, PAD_NEG).astype(np.float32)),
        )

    in_maps = []
    for c in range(2 * B):
        b, hp = c // 2, c % 2
        m = dict(per_hp[hp])
        m.update(per_b[b])
        m["cos_t"] = cos128_bf
        m["sin_t"] = sin128_bf
        m["ones_in"] = _bf16(ones)
        m["ident"] = _bf16(ident)
        m["maskb"] = _bf16(maskb)
        in_maps.append(m)
    return in_maps


def run(x, attention_padding, W_qkv, b_qkv, W_proj, b_proj, trace=False, **spmd_kw):
    x = np.asarray(x, dtype=np.float32)
    B, s, _ = x.shape
    zero_bias = bool(
        np.all(np.asarray(b_qkv) == 0) and np.all(np.asarray(b_proj) == 0)
    )
    nc = _built(s, zero_bias)
    in_maps = make_in_maps(x, attention_padding, W_qkv, b_qkv, W_proj, b_proj)
    res = run_bass_kernel_spmd(nc, in_maps, list(range(2 * B)), trace=trace, **spmd_kw)
    out = np.stack(
        [res.results[2 * b]["z"] + res.results[2 * b + 1]["z"] for b in range(B)]
    ).astype(np.float32)
    return out, res


def kernel(x, attention_padding, W_qkv, b_qkv, W_proj, b_proj, train=None, **_):
    out, _res = run(x, attention_padding, W_qkv, b_qkv, W_proj, b_proj)
    return out


# revision 30
# speedup vs baseline: 1.0273x; 1.0273x over previous
"""Trainium2 Bass kernel for ManualCausalSelfAttention.

Full (unsharded) inputs -> full output. Internally shards across 8
NeuronCores: data-parallel over batch (4) x tensor-parallel over head
groups of 8 heads (2). Each core computes a partial output projection
(its 512 rows of W_proj); the host sums the two partials per batch.

v2 design (vs baseline):
  - all matmul operands bf16 (psum stays f32): halves DMA/SBUF/LDW and
    unlocks DVE 2x modes; fp32r was already 1 cyc/row at N>=256 so PE
    cycles are unchanged.
  - single x streaming pass: q/k projection + RoPE and v projection
    share the same xt tile loop.
  - psum->sbuf bias-add + bf16 cast on the (otherwise idle) Scalar
    engine via activation(Identity, bias=per-partition).
  - causal mask without gpsimd affine_select: sub-diagonal key tiles are
    skipped outright (valid-region ST/exp/PV), and the 128-wide diagonal
    blocks get a -480 bias added in PSUM via a tiny identity x maskbias
    matmul before the exp (exp scale 1/8 turns it into -60).
  - out-proj psum tiles share the ST pool so PSUM fits in 8 banks with
    double-buffered attention pipelines on both the ST and PV side.
"""

import functools
import os
import sys

import numpy as np

for _p in (
    "/root/.axon_site",
    "/root/.axon_site/_ro/trn_rl_repo",
    "/root/.axon_site/_ro/pypackages",
    "/opt/trn_rl_repo",
):
    if os.path.isdir(_p) and _p not in sys.path:
        sys.path.append(_p)

import concourse.bass as bass
import concourse.mybir as mybir
import concourse.tile as tile
from concourse.bass_utils import run_bass_kernel_spmd

# The PJRT boundary swallows python exceptions raised by the bass compile
# hook ("CallFunctionObjArgs: error condition !(py_result)"); wrap the hook
# so the real compile error is printed before being re-raised.
from concourse import bass2jax as _b2j

if not getattr(_b2j, "_hook_wrapped", False):
    _orig_hook = _b2j.neuronx_cc_hook

    def _loud_hook(*a, **k):
        try:
            return _orig_hook(*a, **k)
        except BaseException:
            import traceback

            traceback.print_exc()
            raise

    _b2j.neuronx_cc_hook = _loud_hook
    _b2j._hook_wrapped = True

HID = 1024
NH_GLOBAL = 16
NHL = 8  # heads per core
D = 64
THETA = 10000.0
PAD_NEG = -60.0  # exp(scale*logit + PAD_NEG) == 0 relative to real keys
MASK_NEG = -480.0  # pre-scale causal bias: -480/8 == -60
F32 = mybir.dt.float32
BF16 = mybir.dt.bfloat16
ALU = mybir.AluOpType
AFT = mybir.ActivationFunctionType

TB = 512  # token block (phase 1, matmul N)
QB = 512  # query block (phase 2)

# stream_shuffle mask swapping adjacent partition pairs (per 32-group)
PAIR_SWAP_MASK = [i + 1 - 2 * (i % 2) for i in range(32)]


def _split_sync_waits(nc, max_waits=1):
    """walrus in this environment rejects instructions carrying more than
    `max_waits` sem waits ("Too many sync wait commands"); Tile's kernel-tail
    drain carries one wait per logical proc. Split the excess waits onto
    preceding same-engine NOPs."""
    for fn in nc.m.functions:
        for blk in fn.blocks:
            if not any(
                ins.sync_info is not None
                and ins.sync_info.on_wait
                and len(ins.sync_info.on_wait)
                > (0 if isinstance(ins, mybir.InstDrain) else max_waits)
                for ins in blk.instructions
            ):
                continue
            new_insts = []
            for ins in blk.instructions:
                si = ins.sync_info
                limit = 0 if isinstance(ins, mybir.InstDrain) else max_waits
                if si is not None and si.on_wait and len(si.on_wait) > limit:
                    waits = list(si.on_wait)
                    if limit:
                        extra, keep = waits[:-limit], waits[-limit:]
                    else:
                        extra, keep = waits, []
                    for i in range(0, len(extra), max_waits):
                        new_insts.append(
                            mybir.InstNoOp(
                                name=f"{ins.name}-sw{i}",
                                sync_info=mybir.SyncInfo(
                                    on_wait=extra[i : i + max_waits], on_update=[]
                                ),
                                bass_nofuse=True,
                                engine=ins.engine,
                            )
                        )
                    ins.sync_info = mybir.SyncInfo(
                        on_wait=keep, on_update=list(si.on_update)
                    )
                new_insts.append(ins)
            blk.instructions[:] = new_insts


def build_kernel(s=2048, split_waits=True, zero_bias=False):
    """Build the per-core Bass module (same program on all 8 cores).

    split_waits applies the walrus wait-limit workaround; disable it when
    running under CoreSim (the sim rejects the unregistered NOPs).
    zero_bias skips the bias matmuls (selected host-side when the actual
    b_qkv/b_proj inputs are all zero, as in this problem's setup).
    """
    nc = bass.Bass()
    nb = s // TB      # token blocks
    nqb = s // QB     # query blocks
    nkt = s // 128    # 128-wide key tiles / token tiles
    hkt = HID // 128  # hidden contraction tiles

    xT = nc.dram_tensor("xt", [HID, s], BF16, kind="ExternalInput")
    w_qk = nc.dram_tensor("w_qk", [HID, 2 * NHL * D], BF16, kind="ExternalInput")
    w_v = nc.dram_tensor("w_v", [HID, NHL * D], BF16, kind="ExternalInput")
    w_pr = nc.dram_tensor("w_proj", [NHL * D, HID], BF16, kind="ExternalInput")
    cos_d = nc.dram_tensor("cos_t", [128, s], BF16, kind="ExternalInput")
    sin_d = nc.dram_tensor("sin_t", [128, s], BF16, kind="ExternalInput")
    bqk_d = nc.dram_tensor("b_qk", [128, 8], F32, kind="ExternalInput")
    bv_d = nc.dram_tensor("b_v", [1, NHL * D], BF16, kind="ExternalInput")
    bpr_d = nc.dram_tensor("b_proj", [1, HID], BF16, kind="ExternalInput")
    padm_d = nc.dram_tensor("pad_mul", [128, nkt], F32, kind="ExternalInput")
    padb_d = nc.dram_tensor("pad_bias", [128, nkt], F32, kind="ExternalInput")
    ones_d = nc.dram_tensor("ones_in", [128, 128], BF16, kind="ExternalInput")
    ident_d = nc.dram_tensor("ident", [128, 128], BF16, kind="ExternalInput")
    maskb_d = nc.dram_tensor("maskb", [128, 128], BF16, kind="ExternalInput")
    z_d = nc.dram_tensor("z", [s, HID], F32, kind="ExternalOutput")
    # DRAM bounce buffers: transpose denominators onto 128 partitions for an
    # efficient exact reciprocal, then broadcast the result across partitions.
    den_scr = nc.dram_tensor("den_scr", [nqb * 4, 2 * QB], F32)
    rcp_scr = nc.dram_tensor("rcp_scr", [nqb * 4, 2 * QB], F32)

    with tile.TileContext(nc) as tc:
        with (
            tc.tile_pool(name="persist", bufs=1) as ppool,
            tc.tile_pool(name="qkstore", bufs=1) as qkpool,
            tc.tile_pool(name="vstore", bufs=1) as vpool,
        ):
            padm_sb = ppool.tile([128, nkt], F32, tag="padm")
            padb_sb = ppool.tile([128, nkt], F32, tag="padb")
            ones_sb = ppool.tile([1, 128], BF16, tag="ones")
            ident_sb = ppool.tile([128, 128], BF16, tag="ident")
            maskb_sb = ppool.tile([128, 128], BF16, tag="maskb")
            nc.sync.dma_start(padm_sb[:], padm_d[:])
            nc.sync.dma_start(padb_sb[:], padb_d[:])
            nc.sync.dma_start(ones_sb[:], ones_d[0:1, 0:128])
            nc.sync.dma_start(ident_sb[:], ident_d[:])
            nc.sync.dma_start(maskb_sb[:], maskb_d[:])

            # q'T / k'T (RoPE'd, feature-major): 8 tiles of [128, s];
            # tiles 0..3 = Q head-pairs, 4..7 = K head-pairs.
            qk_store = [
                qkpool.tile([128, s], BF16, name=f"qk{mt}", tag=f"qk{mt}")
                for mt in range(8)
            ]
            # V with the denominator-ones column appended per head. A DMA
            # here would need 16k 2-byte descriptors; memset is ~free.
            v_sb = vpool.tile([128, nkt, NHL, D + 1], BF16, tag="v")
            nc.vector.memset(v_sb[:, :, :, D : D + 1], 1.0)

            # ------- Single software pipeline: per token block nbi, do the
            # q/k/v projection + RoPE for that block, then the attention for
            # query block qb==nbi (causality: it only needs k/v blocks <=
            # nbi), then the out-projection for qb-1 (one block behind so
            # the softmax-denominator DRAM bounce never stalls the PE).
            with (
                tc.tile_pool(name="xt", bufs=2) as xpool,
                tc.tile_pool(name="c1", bufs=1) as c1pool,
                tc.tile_pool(name="ps_big", bufs=2, space="PSUM") as psBig,
                tc.tile_pool(name="ps_pv", bufs=2, space="PSUM") as psPV,
                tc.tile_pool(name="rope", bufs=4) as rpool,
                tc.tile_pool(name="pt", bufs=4) as ptpool,
                tc.tile_pool(name="ytn", bufs=8) as ytnpool,
                tc.tile_pool(name="small", bufs=3) as smpool,
                tc.tile_pool(name="zs", bufs=3) as zspool,
            ):
                w_qk_sb = c1pool.tile([128, hkt, 2 * NHL * D], BF16, tag="wqk")
                w_v_sb = c1pool.tile([128, hkt, NHL * D], BF16, tag="wv")
                cos_sb = c1pool.tile([128, s], BF16, tag="cos")
                sin_sb = c1pool.tile([128, s], BF16, tag="sin")
                bqk_sb = c1pool.tile([128, 8], F32, tag="bqk")
                bv_sb = c1pool.tile([1, NHL * D], BF16, tag="bv")
                w_pr_sb = c1pool.tile([128, NHL * D // 128, HID], BF16, tag="wpr")
                bpr_sb = c1pool.tile([1, HID], BF16, tag="bpr")
                xT_r = xT.rearrange("(kt p) t -> p kt t", p=128)
                xt0 = xpool.tile([128, hkt, TB], BF16, tag="xt")
                for i in range(4):
                    nc.sync.dma_start(
                        xt0[:, 2 * i : 2 * i + 2, :], xT_r[:, 2 * i : 2 * i + 2, 0:TB]
                    )
                # split the big weight DMA so several queues run in parallel;
                # column-group order matches the mt emission order (k first)
                w_qk_r = w_qk.rearrange("(kt p) f -> p kt f", p=128)
                for c in (2, 3, 0, 1):
                    nc.sync.dma_start(
                        w_qk_sb[:, :, c * 256 : (c + 1) * 256],
                        w_qk_r[:, :, c * 256 : (c + 1) * 256],
                    )
                nc.sync.dma_start(
                    w_v_sb[:], w_v.rearrange("(kt p) f -> p kt f", p=128)
                )
                nc.sync.dma_start(cos_sb[:], cos_d[:])
                nc.sync.dma_start(sin_sb[:], sin_d[:])
                nc.sync.dma_start(bqk_sb[:], bqk_d[:])
                nc.sync.dma_start(bv_sb[:], bv_d[:])
                nc.sync.dma_start(
                    w_pr_sb[:], w_pr.rearrange("(sl p) f -> p sl f", p=128)
                )
                nc.sync.dma_start(bpr_sb[:], bpr_d[:])

                def emit_proj(nbi, xt):
                    tsl = slice(nbi * TB, (nbi + 1) * TB)
                    # k head-pairs first (mt 4..7): the first ST matmuls of
                    # this query block depend on k of every block plus q of
                    # pair 0, so this order lets attention start while the
                    # DVE still applies RoPE to q pairs 1-3.
                    for mt in (4, 5, 6, 7, 0, 1, 2, 3):
                        ps = psBig.tile([128, 2 * QB], F32, tag="big")
                        for kt in range(hkt):
                            nc.tensor.matmul(
                                ps[:, 0:TB],
                                w_qk_sb[:, kt, mt * 128 : (mt + 1) * 128],
                                xt[:, kt, :],
                                start=(kt == 0),
                                stop=(kt == hkt - 1),
                            )
                        # psum -> sbuf bf16 with per-partition bias on ScalarE
                        qraw = rpool.tile([128, TB], BF16, tag="qraw")
                        nc.scalar.activation(
                            qraw[:], ps[:, 0:TB], AFT.Identity,
                            bias=0.0 if zero_bias else bqk_sb[:, mt : mt + 1],
                        )
                        t1 = rpool.tile([128, TB], BF16, tag="t1")
                        t2p = rpool.tile([128, TB], BF16, tag="t2p")
                        t2 = rpool.tile([128, TB], BF16, tag="t2")
                        nc.vector.tensor_mul(t1[:], qraw[:], cos_sb[:, tsl])
                        # pair-swapping (qraw*sin_pre) gives rotate_half(qraw)
                        # * sin (sign is in the table: +sin even rows, -sin
                        # odd rows).
                        nc.vector.tensor_mul(t2p[:], qraw[:], sin_sb[:, tsl])
                        nc.vector.stream_shuffle(t2[:], t2p[:], PAIR_SWAP_MASK)
                        nc.vector.tensor_add(qk_store[mt][:, tsl], t1[:], t2[:])
                    for vt in range(TB // 128):
                        psv = psBig.tile([128, 2 * QB], F32, tag="big")
                        for kt in range(hkt):
                            nc.tensor.matmul(
                                psv[:, 0 : NHL * D],
                                xt[:, kt, vt * 128 : (vt + 1) * 128],
                                w_v_sb[:, kt, :],
                                start=(kt == 0),
                                stop=(zero_bias and kt == hkt - 1),
                            )
                        if not zero_bias:
                            nc.tensor.matmul(
                                psv[:, 0 : NHL * D],
                                ones_sb[:],
                                bv_sb[:],
                                start=False,
                                stop=True,
                            )
                        ktix = nbi * (TB // 128) + vt
                        # psum -> sbuf bf16 cast on ScalarE
                        nc.scalar.activation(
                            v_sb[:, ktix, :, 0:D],
                            psv[:, 0 : NHL * D].rearrange("p (h d) -> p h d", d=D),
                            AFT.Identity,
                        )

                def emit_tt_head(qb, ytns, tt):
                    zp = psBig.tile([128, 2 * QB], F32, tag="big")
                    for ob in range(HID // 512):
                        for pair in range(3):
                            nc.tensor.matmul(
                                zp[:, ob * 512 : (ob + 1) * 512],
                                ytns[pair][:, tt * 128 : (tt + 1) * 128],
                                w_pr_sb[:, pair, ob * 512 : (ob + 1) * 512],
                                start=(pair == 0),
                                stop=False,
                            )
                    return zp

                def emit_tt_tail(qb, ytns, tt, zp):
                    gt = qb * (QB // 128) + tt
                    for ob in range(HID // 512):
                        nc.tensor.matmul(
                            zp[:, ob * 512 : (ob + 1) * 512],
                            ytns[3][:, tt * 128 : (tt + 1) * 128],
                            w_pr_sb[:, 3, ob * 512 : (ob + 1) * 512],
                            start=False,
                            stop=zero_bias,
                        )
                        if not zero_bias:
                            nc.tensor.matmul(
                                zp[:, ob * 512 : (ob + 1) * 512],
                                ones_sb[:],
                                bpr_sb[:, ob * 512 : (ob + 1) * 512],
                                start=False,
                                stop=True,
                            )
                    zs = zspool.tile([128, HID], F32, tag="zs")
                    nc.vector.tensor_scalar(
                        out=zs[:],
                        in0=zp[:],
                        scalar1=padm_sb[:, gt : gt + 1],
                        scalar2=None,
                        op0=ALU.mult,
                    )
                    nc.sync.dma_start(z_d[gt * 128 : (gt + 1) * 128, :], zs[:])

                def emit_outproj(qb, ytns, batch_tail=False):
                    # out-projection for query block qb (runs one qb behind
                    # attention so the den-reciprocal DRAM bounce never
                    # stalls the in-order PE). batch_tail (final block only,
                    # where this IS the critical path): emit the pair-0..2
                    # matmuls of two token tiles before any pair-3 matmul,
                    # so most work issues while pair 3's normalization is
                    # still in flight.
                    if not batch_tail:
                        for tt in range(QB // 128):
                            zp = emit_tt_head(qb, ytns, tt)
                            emit_tt_tail(qb, ytns, tt, zp)
                    else:
                        for tg in range(0, QB // 128, 2):
                            zp0 = emit_tt_head(qb, ytns, tg)
                            zp1 = emit_tt_head(qb, ytns, tg + 1)
                            emit_tt_tail(qb, ytns, tg, zp0)
                            emit_tt_tail(qb, ytns, tg + 1, zp1)

                prev = None
                xt_next = xt0
                for nbi in range(nb):
                    qb = nbi
                    xt = xt_next
                    emit_proj(nbi, xt)
                    if nbi + 1 < nb:
                        # prefetch next token block while attention runs
                        xt_next = xpool.tile([128, hkt, TB], BF16, tag="xt")
                        nc.sync.dma_start(
                            xt_next[:],
                            xT_r[:, :, (nbi + 1) * TB : (nbi + 2) * TB],
                        )
                    nkts = (qb + 1) * (QB // 128)
                    ytns = []
                    for pair in range(4):
                        qst = qk_store[pair]
                        kst = qk_store[4 + pair]
                        yp = psPV.tile([D + 1, 2 * QB], F32, name="pv", tag="pv")
                        dstart = qb * (QB // 128)
                        for ki, kt in enumerate(range(nkts)):
                            r = kt - dstart  # >=0 on diagonal group
                            off = max(r, 0) * 128
                            stp = psBig.tile([128, 2 * QB], F32, tag="big")
                            for h2 in (0, 1):
                                lo = h2 * 64
                                nc.tensor.matmul(
                                    stp[:, h2 * QB + off : (h2 + 1) * QB],
                                    kst[lo : lo + 64, kt * 128 : (kt + 1) * 128],
                                    qst[lo : lo + 64, qb * QB + off : (qb + 1) * QB],
                                    start=True,
                                    stop=(r < 0),
                                    tile_position=(lo, 0),
                                )
                                if r >= 0:
                                    # diagonal 128-block: add -480 causal bias
                                    nc.tensor.matmul(
                                        stp[:, h2 * QB + off : h2 * QB + off + 128],
                                        ident_sb[:],
                                        maskb_sb[:],
                                        start=False,
                                        stop=True,
                                    )
                            # P = exp(logit/8 + pad_bias_k) on the valid cols
                            pt = ptpool.tile([128, 2, QB], BF16, tag="pt")
                            nc.scalar.activation(
                                pt[:, :, off:],
                                stp.rearrange("p (h q) -> p h q", h=2)[:, :, off:],
                                AFT.Exp,
                                bias=padb_sb[:, kt : kt + 1],
                                scale=float(D) ** -0.5,
                            )
                            for h2 in (0, 1):
                                head = pair * 2 + h2
                                nc.tensor.matmul(
                                    yp[:, h2 * QB + off : (h2 + 1) * QB],
                                    v_sb[:, kt, head, :],
                                    pt[:, h2, off:],
                                    start=(ki == 0),
                                    stop=(ki == nkts - 1),
                                )
                        # Softmax denominators: bounce the accumulator row
                        # through DRAM to spread it over 128 partitions, take
                        # an exact DVE reciprocal, bounce back, and broadcast
                        # per head.
                        row = qb * 4 + pair
                        den = smpool.tile([1, 2 * QB], F32, name="den", tag="den")
                        nc.vector.tensor_copy(den[:], yp[D : D + 1, :])
                        nc.sync.dma_start(den_scr[row : row + 1, :], den[:])
                        dent = smpool.tile(
                            [128, 2 * QB // 128], F32, name="dent", tag="dent"
                        )
                        nc.sync.dma_start(
                            dent[:],
                            den_scr[row : row + 1, :].rearrange(
                                "o (p f) -> (o p) f", p=128
                            ),
                        )
                        rcpt = smpool.tile(
                            [128, 2 * QB // 128], F32, name="rcpt", tag="rcpt"
                        )
                        nc.vector.reciprocal(rcpt[:], dent[:])
                        nc.sync.dma_start(
                            rcp_scr[row : row + 1, :].rearrange(
                                "o (p f) -> (o p) f", p=128
                            ),
                            rcpt[:],
                        )
                        ytn = ytnpool.tile([128, QB], BF16, tag="ytn")
                        for h2 in (0, 1):
                            rb = smpool.tile([64, QB], F32, tag="rb")
                            nc.sync.dma_start(
                                rb[:],
                                rcp_scr[
                                    row : row + 1, h2 * QB : (h2 + 1) * QB
                                ].broadcast_to([64, QB]),
                            )
                            nc.vector.tensor_mul(
                                ytn[h2 * 64 : (h2 + 1) * 64, :],
                                yp[0:D, h2 * QB : (h2 + 1) * QB],
                                rb[:],
                            )
                        ytns.append(ytn)
                        if pair == 1 and prev is not None:
                            # out-proj of the previous query block, emitted
                            # mid-attention: its operands are long ready, so
                            # it fills PE while the exp pipeline works.
                            emit_outproj(qb - 1, prev)
                    prev = ytns
                emit_outproj(nqb - 1, prev, batch_tail=True)

                # (loop above assumes one query block per token block)
                assert nb == nqb
    if split_waits:
        _split_sync_waits(nc)
    return nc


@functools.lru_cache(maxsize=2)
def _built(s, zero_bias=False):
    return build_kernel(s, zero_bias=zero_bias)


def _rope_tables(s):
    j = np.arange(D // 2, dtype=np.float64)
    inv = THETA ** (-2.0 * j / D)
    ang = np.arange(s, dtype=np.float64)[:, None] * inv[None, :]  # [s, 32]
    cos = np.cos(ang).T  # [32, s]
    sin = np.sin(ang).T
    cos64 = np.repeat(cos, 2, axis=0)  # rows 2j, 2j+1 identical
    sin64 = np.repeat(sin, 2, axis=0)
    # "pre-swap" sign convention: the kernel multiplies by this table BEFORE
    # pair-swapping partitions, so odd rows carry the minus sign.
    sin64[1::2, :] *= -1.0
    cos128 = np.concatenate([cos64, cos64], axis=0)
    sin128 = np.concatenate([sin64, sin64], axis=0)
    return np.ascontiguousarray(cos128), np.ascontiguousarray(sin128)


def _col_tiled(vec, tile_rows=128):
    """[n] -> [tile_rows, n//tile_rows], column t = vec[t*128:(t+1)*128]."""
    n = vec.shape[0]
    return np.ascontiguousarray(vec.reshape(n // tile_rows, tile_rows).T)


def _bf16(a):
    import ml_dtypes

    return np.ascontiguousarray(np.asarray(a).astype(ml_dtypes.bfloat16))


def make_in_maps(x, attention_padding, W_qkv, b_qkv, W_proj, b_proj):
    x = np.asarray(x, dtype=np.float32)
    pad = np.asarray(attention_padding).astype(bool)
    W_qkv = np.asarray(W_qkv, dtype=np.float32)
    b_qkv = np.asarray(b_qkv, dtype=np.float32)
    W_proj = np.asarray(W_proj, dtype=np.float32)
    b_proj = np.asarray(b_proj, dtype=np.float32)
    B, s, _ = x.shape
    cos128, sin128 = _rope_tables(s)
    cos128_bf = _bf16(cos128)
    sin128_bf = _bf16(sin128)
    ident = np.eye(128, dtype=np.float32)
    ones = np.ones((128, 128), dtype=np.float32)
    maskb = np.where(
        np.arange(128)[None, :] < np.arange(128)[:, None], MASK_NEG, 0.0
    ).astype(np.float32)

    per_hp = {}
    for hp in range(2):
        hs = slice(hp * NHL * D, (hp + 1) * NHL * D)
        Wq = W_qkv[:, 0:HID][:, hs]
        Wk = W_qkv[:, HID : 2 * HID][:, hs]
        Wv = W_qkv[:, 2 * HID : 3 * HID][:, hs]
        bq = b_qkv[0:HID][hs]
        bk = b_qkv[HID : 2 * HID][hs]
        bv = b_qkv[2 * HID : 3 * HID][hs]
        bqk = np.concatenate([bq, bk])
        per_hp[hp] = dict(
            w_qk=_bf16(np.concatenate([Wq, Wk], axis=1)),
            w_v=_bf16(Wv),
            w_proj=_bf16(W_proj[hs, :]),
            b_qk=_col_tiled(bqk),
            b_v=_bf16(bv[None, :]),
            b_proj=_bf16(
                (b_proj if hp == 0 else np.zeros_like(b_proj))[None, :]
            ),
        )

    per_b = {}
    for b in range(B):
        p = pad[b].astype(np.float32)
        per_b[b] = dict(
            xt=_bf16(x[b].T),
            pad_mul=_col_tiled(p),
            pad_bias=_col_tiled(np.where(pad[b], 0.0, PAD_NEG).astype(np.float32)),
        )

    in_maps = []
    for c in range(2 * B):
        b, hp = c // 2, c % 2
        m = dict(per_hp[hp])
        m.update(per_b[b])
        m["cos_t"] = cos128_bf
        m["sin_t"] = sin128_bf
        m["ones_in"] = _bf16(ones)
        m["ident"] = _bf16(ident)
        m["maskb"] = _bf16(maskb)
        in_maps.append(m)
    return in_maps


def run(x, attention_padding, W_qkv, b_qkv, W_proj, b_proj, trace=False, **spmd_kw):
    x = np.asarray(x, dtype=np.float32)
    B, s, _ = x.shape
    zero_bias = bool(
        np.all(np.asarray(b_qkv) == 0) and np.all(np.asarray(b_proj) == 0)
    )
    nc = _built(s, zero_bias)
    in_maps = make_in_maps(x, attention_padding, W_qkv, b_qkv, W_proj, b_proj)
    res = run_bass_kernel_spmd(nc, in_maps, list(range(2 * B)), trace=trace, **spmd_kw)
    out = np.stack(
        [res.results[2 * b]["z"] + res.results[2 * b + 1]["z"] for b in range(B)]
    ).astype(np.float32)
    return out, res


def kernel(x, attention_padding, W_qkv, b_qkv, W_proj, b_proj, train=None, **_):
    out, _res = run(x, attention_padding, W_qkv, b_qkv, W_proj, b_proj)
    return out
